# revision 3
# baseline (speedup 1.0000x reference)
"""Distributed masked-attention kernel for 8 TRN2 NeuronCores (v4).

Reference computation (B=2, L=1024, D=1024, H=16, DH=64):
    Qz, Kz = masked Q, K;  Qp/Kp/Vp = projections (V = K)
    per-head attention with outer-product validity mask, softmax scaled
    by 1/sqrt(D);  O = Qp + attn;  out = O + relu(mask_q(O @ Wo.T))

Sharding: core c = 2*g + b handles batch b = c%2, head group g = c//2
(4 heads, feature block e = [256g, 256g+256)).  Activations are
feature-major ("X.T" = [features, tokens]); host pre-transposes,
pre-zeroes masked rows, and casts to bf16.

v4 structure (vs v3):
  - projections accumulate in 8 PSUM banks and consume input-DMA chunks
    as they arrive (dc-pipelined) so the PE stays dense from ~3us.
  - softmax denominators: DVE reciprocal straight from PSUM (ACT only
    does exp); epilogue relu+residual fused into one DVE
    scalar_tensor_tensor.
  - 4 smaller AllToAlls (one per (q-chunk, feature-half), 128KB) fire
    as soon as each feature-half's residual is done.
  - epilogue passes contract even-dc blocks first so pass 1 only tails
    on the last A2A by its odd-dc half.
"""
import numpy as np

B, L, D = 2, 1024, 1024
H, DH = 16, 64
NCORES = 8
HPC = 4          # heads per core
EB = 256         # feature block per core
NEG = -30000.0   # masked-k bias (exp -> exact 0)
BIG = 1e30       # masked-q denominator prefill

TRACE = False
TRACE_KWARGS = {}
LAST_RESULTS = None

_compiled = None


def _build():
    import concourse.bacc as bacc
    import concourse.tile as tile
    from concourse.tile import add_dep_helper
    from concourse import mybir

    f32 = mybir.dt.float32
    bf16 = mybir.dt.bfloat16
    EXP = mybir.ActivationFunctionType.Exp
    ALU = mybir.AluOpType

    nc = bacc.Bacc("TRN2", target_bir_lowering=False, debug=False,
                   num_devices=NCORES)

    qt = nc.dram_tensor("qt", [D, L], bf16, kind="ExternalInput")
    kt = nc.dram_tensor("kt", [D, L], bf16, kind="ExternalInput")
    wq = nc.dram_tensor("wq", [D, EB], bf16, kind="ExternalInput")
    wk = nc.dram_tensor("wk", [D, EB], bf16, kind="ExternalInput")
    wv = nc.dram_tensor("wv", [D, EB], bf16, kind="ExternalInput")
    wo = nc.dram_tensor("wo", [D, D], bf16, kind="ExternalInput")
    bk = nc.dram_tensor("bk", [128, 8], f32, kind="ExternalInput")
    e64 = nc.dram_tensor("e64", [1, 65], bf16, kind="ExternalInput")
    mvn = nc.dram_tensor("mvn", [1, L], bf16, kind="ExternalInput")
    out = nc.dram_tensor("out", [D, 256], f32, kind="ExternalOutput")

    with tile.TileContext(nc) as tc:
        with (
            tc.tile_pool(name="sb", bufs=1) as sb,
            tc.tile_pool(name="rot", bufs=4) as rot,
            tc.tile_pool(name="ps_big", bufs=2, space="PSUM") as ps_big,
            tc.tile_pool(name="ps_at", bufs=4, space="PSUM") as ps_at,
            tc.tile_pool(name="dram", bufs=1, space="DRAM") as dram,
        ):
            # ---- dummy collective first: absorbs cross-core start skew
            # and ncfw cold-start while inputs stream in ----
            dumin = dram.tile([8, 32], f32, tag="dumin", name="dumin")
            dumout = dram.tile([64, 32], f32, tag="dumout", name="dumout")
            dum_sb = sb.tile([8, 32], f32, tag="dumsb")
            nc.gpsimd.memset(dum_sb[:], 1.0)
            nc.gpsimd.dma_start(dumin[:], dum_sb[:])
            nc.gpsimd.collective_compute(
                "AllGather", ALU.bypass,
                replica_groups=[list(range(NCORES))],
                ins=[dumin[:].opt()], outs=[dumout[:].opt()])

            # ---- constants / masks (tiny) ----
            bk_t = sb.tile([128, 8], f32, tag="bk")
            e64_t = sb.tile([1, 65], bf16, tag="e64")
            mvn_t = sb.tile([1, L], bf16, tag="mvn")
            nc.sync.dma_start(bk_t[:], bk[:])
            nc.sync.dma_start(e64_t[:], e64[:])
            nc.sync.dma_start(mvn_t[:], mvn[:])

            # ---- warmup: PE matmuls + ACT exp-table load, no input deps ----
            warm_w = sb.tile([128, 128], bf16, tag="warmw")
            warm_x = sb.tile([128, 512], bf16, tag="warmx")
            nc.vector.memset(warm_w[:], 0.5)
            nc.vector.memset(warm_x[:], 0.5)
            warm_act = rot.tile([128, 512], bf16, tag="p", name="warm_act")
            nc.scalar.activation(warm_act[:], warm_x[:], EXP, scale=0.25)
            warm_ps = ps_big.tile([128, 1024], f32, tag="big", name="warm_ps")
            for w in range(9):
                nc.tensor.matmul(warm_ps[:, 0:512], warm_w[:], warm_x[:],
                                 start=(w == 0), stop=(w == 8))

            # ---- input DMAs: dc-pair chunks on the two HWDGE queues ----
            def pair_load(eng, dram_t, F, i, tagp):
                t2 = sb.tile([128, 2 * F], bf16, tag=f"{tagp}{i}",
                             name=f"{tagp}{i}")
                dst = t2[:].rearrange("p (h t) -> p h t", h=2)
                srcv = dram_t[256 * i:256 * (i + 1), :].rearrange(
                    "(h p) t -> p h t", p=128)
                eng.dma_start(dst, srcv)
                return t2

            qt_2, kt_2, wq_2, wk_2, wv_2 = [], [], [], [], []
            for i in range(4):
                wq_2.append(pair_load(nc.sync, wq, EB, i, "wq"))
                qt_2.append(pair_load(nc.sync, qt, L, i, "qt"))
                wk_2.append(pair_load(nc.scalar, wk, EB, i, "wk"))
                kt_2.append(pair_load(nc.scalar, kt, L, i, "kt"))
                wv_2.append(pair_load(nc.gpsimd, wv, EB, i, "wv"))
            qt_t = [qt_2[i // 2][:, L * (i % 2):L * (i % 2 + 1)]
                    for i in range(8)]
            kt_t = [kt_2[i // 2][:, L * (i % 2):L * (i % 2 + 1)]
                    for i in range(8)]
            wq_t = [wq_2[i // 2][:, EB * (i % 2):EB * (i % 2 + 1)]
                    for i in range(8)]
            wk_t = [wk_2[i // 2][:, EB * (i % 2):EB * (i % 2 + 1)]
                    for i in range(8)]
            wv_t = [wv_2[i // 2][:, EB * (i % 2):EB * (i % 2 + 1)]
                    for i in range(8)]

            # ---- phase 1: projections, dc-pipelined, PSUM-resident ----
            # 8 banks: QP0/KP0 = [128,1024] (qc-concat, et0) in ps_big,
            # QP1/KP1 as 4x [128,512] (et1) in ps_at.
            qp0 = ps_big.tile([128, 1024], f32, tag="big", name="qp0")
            kp0 = ps_big.tile([128, 1024], f32, tag="big", name="kp0")
            qp1 = [ps_at.tile([128, 512], f32, tag="at", name=f"qp1_{qc}")
                   for qc in range(2)]
            kp1 = [ps_at.tile([128, 512], f32, tag="at", name=f"kp1_{qc}")
                   for qc in range(2)]
            for dc in range(8):
                st = (dc == 0)
                sp = (dc == 7)
                for qc in range(2):
                    qs = slice(512 * qc, 512 * (qc + 1))
                    nc.tensor.matmul(qp0[:, qs], wq_t[dc][:, 0:128],
                                     qt_t[dc][:, qs], start=st, stop=sp)
                for qc in range(2):
                    qs = slice(512 * qc, 512 * (qc + 1))
                    nc.tensor.matmul(qp1[qc][:], wq_t[dc][:, 128:256],
                                     qt_t[dc][:, qs], start=st, stop=sp)
                for qc in range(2):
                    qs = slice(512 * qc, 512 * (qc + 1))
                    nc.tensor.matmul(kp0[:, qs], wk_t[dc][:, 0:128],
                                     kt_t[dc][:, qs], start=st, stop=sp)
                for qc in range(2):
                    qs = slice(512 * qc, 512 * (qc + 1))
                    nc.tensor.matmul(kp1[qc][:], wk_t[dc][:, 128:256],
                                     kt_t[dc][:, qs], start=st, stop=sp)

            # PSUM -> SBUF (inputs pre-zeroed on host, plain copies)
            qpt = [sb.tile([128, L], bf16, tag=f"qpt{i}", name=f"qpt{i}")
                   for i in range(2)]
            kpt = [sb.tile([128, L], bf16, tag=f"kpt{i}", name=f"kpt{i}")
                   for i in range(2)]
            cp_et0 = nc.vector.tensor_copy(kpt[0][:, :], kp0[:])
            nc.vector.tensor_copy(qpt[0][:, :], qp0[:])
            for qc in range(2):
                qs = slice(512 * qc, 512 * (qc + 1))
                nc.vector.tensor_copy(kpt[1][:, qs], kp1[qc][:])
                nc.vector.tensor_copy(qpt[1][:, qs], qp1[qc][:])

            # Vp natural [k-tokens, e] with ones column per head (65-stride)
            vpa = [sb.tile([128, 65 * HPC], bf16, tag=f"vpa{i}",
                           name=f"vpa{i}") for i in range(8)]
            for tt in range(8):
                nc.gpsimd.memset(vpa[tt][:], 1.0)

            def vproj(tt):
                pv = ps_at.tile([128, EB], f32, tag="at", name=f"pv{tt}")
                for dc in range(8):
                    nc.tensor.matmul(
                        pv[:], kt_t[dc][:, 128 * tt:128 * (tt + 1)],
                        wv_t[dc][:], start=(dc == 0), stop=(dc == 7))
                for h in range(HPC):
                    nc.vector.tensor_copy(
                        vpa[tt][:, 65 * h:65 * h + 64],
                        pv[:, 64 * h:64 * (h + 1)])

            vproj(0)
            vproj(1)

            # ---- epilogue weights: after input loads (pinned below) ----
            wo_t = [sb.tile([128, D], bf16, tag=f"wo{i}", name=f"wo{i}")
                    for i in range(8)]
            wo_dmas = []
            for i in range(8):
                wo_dmas.append(
                    nc.sync.dma_start(wo_t[i][:], wo[128 * i:128 * (i + 1), :]))
            for dma in wo_dmas:
                add_dep_helper(dma.ins, cp_et0.ins, sync=True,
                               reason="defer wo until proj done")

            # ---- A2A buffers: one per (qc, et), 128KB each ----
            # inb[2qc+et] rows (j recv * 128 feat), cols 64 tokens.
            inb = [dram.tile([1024, 64], bf16, tag=f"inb{i}", name=f"inb{i}")
                   for i in range(4)]
            outb = [dram.tile([1024, 64], bf16, tag=f"outb{i}",
                              name=f"outb{i}") for i in range(4)]
            attn = [sb.tile([128, L], bf16, tag=f"attn{i}", name=f"attn{i}")
                    for i in range(2)]
            ot = [sb.tile([128, L], bf16, tag=f"ot{i}", name=f"ot{i}")
                  for i in range(2)]
            # ot_sl[dt] columns: [qc0-b0 | qc0-b1 | qc1-b0 | qc1-b1], 64 each
            ot_sl = [sb.tile([128, 256], bf16, tag=f"osl{i}", name=f"osl{i}")
                     for i in range(8)]

            def readback(qc, et, eng):
                # outb rows (s=2g+b senders * 128p) -> ot_sl[2g+et]
                for g in range(4):
                    dt_ = 2 * g + et
                    srcv = outb[2 * qc + et][:].rearrange(
                        "(g b p) c -> g b p c", g=4, b=2)[g]
                    srcv = srcv.rearrange("b p c -> p b c")
                    dst = ot_sl[dt_][:, 128 * qc:128 * (qc + 1)].rearrange(
                        "p (b c) -> p b c", b=2)
                    eng.dma_start(dst, srcv)

            # ---- phase 2: attention, q-chunk-major ----
            for qc in range(2):
                qs = slice(512 * qc, 512 * (qc + 1))
                for hp in (0, 2):
                    et = hp // 2
                    ats = []
                    for h in (hp, hp + 1):
                        at = ps_at.tile([65, 512], f32, tag="at",
                                        name=f"at{h}_{qc}")
                        nc.tensor.matmul(at[:], e64_t[:], mvn_t[:, qs],
                                         start=True, stop=False)
                        ats.append(at)
                    # software pipeline: S(k) issued ahead of attn(k-1)
                    p_prev = None
                    for ki in range(8):
                        ks = slice(128 * ki, 128 * (ki + 1))
                        s_ps = ps_big.tile([128, 1024], f32, tag="big",
                                           name=f"s{hp}_{qc}_{ki}")
                        for j in range(2):
                            ro = 64 * j
                            nc.tensor.matmul(
                                s_ps[:, 512 * j:512 * (j + 1)],
                                kpt[et][ro:ro + 64, ks],
                                qpt[et][ro:ro + 64, qs],
                                start=True, stop=True)
                        p_t = rot.tile([128, 1024], bf16, tag="p",
                                       name=f"p{hp}_{qc}_{ki}")
                        nc.scalar.activation(p_t[:], s_ps[:], EXP,
                                             bias=bk_t[:, ki:ki + 1],
                                             scale=1.0 / 32)
                        if qc == 0 and hp == 0 and ki < 6:
                            vproj(ki + 2)
                        if p_prev is not None:
                            kp_, pp = p_prev
                            for j, h in enumerate((hp, hp + 1)):
                                nc.tensor.matmul(
                                    ats[j][:], vpa[kp_][:, 65 * h:65 * h + 65],
                                    pp[:, 512 * j:512 * (j + 1)],
                                    start=False, stop=False)
                        p_prev = (ki, p_t)
                    kp_, pp = p_prev
                    for j, h in enumerate((hp, hp + 1)):
                        nc.tensor.matmul(
                            ats[j][:], vpa[kp_][:, 65 * h:65 * h + 65],
                            pp[:, 512 * j:512 * (j + 1)],
                            start=False, stop=True)
                    # normalize: attn = at[0:64] / denom (denom row 64)
                    # NB: reciprocal_approx_fast is broken on PSUM reads at
                    # partition offset 64 — bounce the row through SBUF.
                    for j, h in enumerate((hp, hp + 1)):
                        ro = 64 * (h % 2)
                        den = rot.tile([1, 512], f32, tag="den")
                        nc.scalar.activation(
                            den[:], ats[j][64:65, :],
                            mybir.ActivationFunctionType.Copy)
                        rcp = rot.tile([1, 512], f32, tag="rcp")
                        nc.vector.reciprocal_approx_fast(rcp[:], den[:])
                        bc = rot.tile([64, 512], f32, tag="bc")
                        nc.gpsimd.partition_broadcast(bc[:], rcp[:])
                        nc.vector.tensor_mul(
                            attn[et][ro:ro + 64, qs], ats[j][0:64, :], bc[:])
                    # residual + bounce + per-(qc,et) A2A
                    nc.vector.tensor_add(ot[et][:, qs], qpt[et][:, qs],
                                         attn[et][:, qs])
                    srcv2 = ot[et][:, qs].rearrange("p (j c) -> p j c", j=8)
                    dstv2 = inb[2 * qc + et][:].rearrange(
                        "(j p) c -> j p c", j=8)
                    nc.gpsimd.dma_start(dstv2.rearrange("j p c -> p j c"),
                                        srcv2)
                    nc.gpsimd.collective_compute(
                        "AllToAll", ALU.bypass,
                        replica_groups=[list(range(NCORES))],
                        ins=[inb[2 * qc + et][:].opt()],
                        outs=[outb[2 * qc + et][:].opt()])
                    readback(qc, et, nc.scalar if et else nc.sync)

            # ---- phase 3: output projection, pass per q-chunk ----
            # even-dc blocks contract first so pass p only tails on the
            # odd-et A2A by its odd-dc half.
            DC_ORDER = [0, 2, 4, 6, 1, 3, 5, 7]
            for p in range(2):
                o_big = sb.tile([128, 1024], f32, tag=f"obig{p}",
                                name=f"obig{p}")
                for et_o in range(8):
                    fpt = ps_at.tile([128, 128], f32, tag="at",
                                     name=f"fp{p}_{et_o}")
                    for n, dc in enumerate(DC_ORDER):
                        nc.tensor.matmul(
                            fpt[:], wo_t[dc][:, 128 * et_o:128 * (et_o + 1)],
                            ot_sl[dc][:, 128 * p:128 * (p + 1)],
                            start=(n == 0), stop=(n == 7))
                    # fused relu+residual: o = max(fpt,0) + ot_sl
                    nc.vector.scalar_tensor_tensor(
                        o_big[:, 128 * et_o:128 * (et_o + 1)],
                        fpt[:], 0.0,
                        ot_sl[et_o][:, 128 * p:128 * (p + 1)],
                        ALU.max, ALU.add)
                # four strided DMAs per pass
                for hf in range(4):
                    dstv = out[256 * hf:256 * (hf + 1),
                               128 * p:128 * (p + 1)].rearrange(
                        "(et p2) c -> p2 et c", p2=128)
                    srcv = o_big[:, 256 * hf:256 * (hf + 1)].rearrange(
                        "p2 (et c) -> p2 et c", et=2)
                    nc.sync.dma_start(dstv, srcv)

    nc.compile()
    return nc


def _get_compiled():
    global _compiled
    if _compiled is None:
        _compiled = _build()
    return _compiled


def kernel(Q, K, mask_Q, mask_K, Wq, Wk, Wv, Wo):
    global LAST_RESULTS
    import ml_dtypes
    from concourse.bass_utils import run_bass_kernel_spmd

    bf = ml_dtypes.bfloat16
    Q = np.asarray(Q, np.float32)
    K = np.asarray(K, np.float32)
    mask_Q = np.asarray(mask_Q, bool)
    mask_K = np.asarray(mask_K, bool)
    Wq = np.asarray(Wq, np.float32)
    Wk = np.asarray(Wk, np.float32)
    Wv = np.asarray(Wv, np.float32)
    Wo = np.asarray(Wo, np.float32)

    nc = _get_compiled()

    # host-side masking: zero masked token rows before projection
    Qz = np.where(mask_Q[:, :, None], 0.0, Q)
    Kz = np.where(mask_K[:, :, None], 0.0, K)

    e64v = np.zeros((1, 65), np.float32)
    e64v[0, 64] = BIG
    wot = np.ascontiguousarray(Wo.T.astype(bf))
    in_maps = []
    for c in range(NCORES):
        b, g = c % 2, c // 2
        eb = slice(EB * g, EB * (g + 1))
        bias = np.where(mask_K[b], NEG, 0.0).astype(np.float32)
        in_maps.append({
            "qt": np.ascontiguousarray(Qz[b].T.astype(bf)),
            "kt": np.ascontiguousarray(Kz[b].T.astype(bf)),
            "wq": np.ascontiguousarray(Wq[eb, :].T.astype(bf)),
            "wk": np.ascontiguousarray(Wk[eb, :].T.astype(bf)),
            "wv": np.ascontiguousarray(Wv[eb, :].T.astype(bf)),
            "wo": wot,
            "bk": np.ascontiguousarray(bias.reshape(8, 128).T),
            "e64": e64v.astype(bf),
            "mvn": mask_Q[b].astype(bf)[None, :],
        })

    res = run_bass_kernel_spmd(nc, in_maps, core_ids=list(range(NCORES)),
                               trace=TRACE, **TRACE_KWARGS)
    LAST_RESULTS = res

    full = np.empty((B, L, D), np.float32)
    for c in range(NCORES):
        o = res.results[c]["out"]   # [1024 e, 256]: [qc0-b0|qc0-b1|qc1-b0|qc1-b1]
        full[0, 64 * c:64 * (c + 1), :] = o[:, 0:64].T
        full[1, 64 * c:64 * (c + 1), :] = o[:, 64:128].T
        full[0, 512 + 64 * c:512 + 64 * (c + 1), :] = o[:, 128:192].T
        full[1, 512 + 64 * c:512 + 64 * (c + 1), :] = o[:, 192:256].T
    return full


# revision 4
# speedup vs baseline: 1.4537x; 1.4537x over previous
"""Distributed masked-attention kernel for 8 TRN2 NeuronCores (v5).

Reference computation (B=2, L=1024, D=1024, H=16, DH=64):
    Qz, Kz = masked Q, K;  Qp/Kp/Vp = projections (V = K)
    per-head attention with outer-product validity mask, softmax scaled
    by 1/sqrt(D);  O = Qp + attn;  out = O + relu(mask_q(O @ Wo.T))

Sharding: core c = 2*g + b handles batch b = c%2, head group g = c//2
(4 heads, feature block e = [256g, 256g+256)).  Activations are
feature-major ("X.T" = [features, tokens]); host pre-transposes,
pre-zeroes masked rows, and casts to bf16.

v5: no collectives.  Each core emits its O block (for the residual)
and its partial output projection O_c @ Wo[:, block].T; the host sums
the four feature-block partials per batch and applies relu + residual
while unsharding.  This removes the cross-core barrier (~50us of ncfw
startup) and the serial AllToAll chain from the device critical path.

  - projections accumulate in 8 PSUM banks and consume input-DMA
    chunks as they arrive (dc-pipelined).
  - ACT does only exp + denominator-row copies; reciprocals on DVE.
  - partial output projection per q-chunk (16 MMs) overlaps the next
    attention chunk; outputs stream out as they are produced.
"""
import numpy as np

B, L, D = 2, 1024, 1024
H, DH = 16, 64
NCORES = 8
HPC = 4          # heads per core
EB = 256         # feature block per core
NEG = -30000.0   # masked-k bias (exp -> exact 0)
BIG = 1e30       # masked-q denominator prefill

TRACE = False
TRACE_KWARGS = {}
LAST_RESULTS = None

_compiled = None


def _build():
    import concourse.bacc as bacc
    import concourse.tile as tile
    from concourse import mybir

    f32 = mybir.dt.float32
    bf16 = mybir.dt.bfloat16
    EXP = mybir.ActivationFunctionType.Exp
    CPY = mybir.ActivationFunctionType.Copy

    nc = bacc.Bacc("TRN2", target_bir_lowering=False, debug=False,
                   num_devices=NCORES)

    qt = nc.dram_tensor("qt", [D, L], bf16, kind="ExternalInput")
    kt = nc.dram_tensor("kt", [D, L], bf16, kind="ExternalInput")
    wq = nc.dram_tensor("wq", [D, EB], bf16, kind="ExternalInput")
    wk = nc.dram_tensor("wk", [D, EB], bf16, kind="ExternalInput")
    wv = nc.dram_tensor("wv", [D, EB], bf16, kind="ExternalInput")
    wos = nc.dram_tensor("wos", [EB, D], bf16, kind="ExternalInput")
    bk = nc.dram_tensor("bk", [128, 8], f32, kind="ExternalInput")
    e64 = nc.dram_tensor("e64", [1, 65], bf16, kind="ExternalInput")
    mvn = nc.dram_tensor("mvn", [1, L], bf16, kind="ExternalInput")
    pout = nc.dram_tensor("pout", [D, L], bf16, kind="ExternalOutput")
    oout = nc.dram_tensor("oout", [EB, L], bf16, kind="ExternalOutput")

    with tile.TileContext(nc) as tc:
        with (
            tc.tile_pool(name="sb", bufs=1) as sb,
            tc.tile_pool(name="rot", bufs=4) as rot,
            tc.tile_pool(name="ps_big", bufs=2, space="PSUM") as ps_big,
            tc.tile_pool(name="ps_at", bufs=4, space="PSUM") as ps_at,
        ):
            # ---- constants / masks (tiny) ----
            bk_t = sb.tile([128, 8], f32, tag="bk")
            e64_t = sb.tile([1, 65], bf16, tag="e64")
            mvn_t = sb.tile([1, L], bf16, tag="mvn")
            nc.sync.dma_start(bk_t[:], bk[:])
            nc.sync.dma_start(e64_t[:], e64[:])
            nc.sync.dma_start(mvn_t[:], mvn[:])

            # ---- warmup: PE matmuls + ACT exp-table load, no input deps ----
            warm_w = sb.tile([128, 128], bf16, tag="warmw")
            warm_x = sb.tile([128, 512], bf16, tag="warmx")
            nc.vector.memset(warm_w[:], 0.5)
            nc.vector.memset(warm_x[:], 0.5)
            warm_act = rot.tile([128, 512], bf16, tag="p", name="warm_act")
            nc.scalar.activation(warm_act[:], warm_x[:], EXP, scale=0.25)
            warm_ps = ps_big.tile([128, 1024], f32, tag="big", name="warm_ps")
            for w in range(9):
                nc.tensor.matmul(warm_ps[:, 0:512], warm_w[:], warm_x[:],
                                 start=(w == 0), stop=(w == 8))

            # ---- input DMAs: dc-pair chunks on the two HWDGE queues ----
            def pair_load(eng, dram_t, F, i, tagp):
                t2 = sb.tile([128, 2 * F], bf16, tag=f"{tagp}{i}",
                             name=f"{tagp}{i}")
                dst = t2[:].rearrange("p (h t) -> p h t", h=2)
                srcv = dram_t[256 * i:256 * (i + 1), :].rearrange(
                    "(h p) t -> p h t", p=128)
                eng.dma_start(dst, srcv)
                return t2

            qt_2, kt_2, wq_2, wk_2, wv_2 = [], [], [], [], []
            for i in range(4):
                wq_2.append(pair_load(nc.sync, wq, EB, i, "wq"))
                qt_2.append(pair_load(nc.sync, qt, L, i, "qt"))
                wk_2.append(pair_load(nc.scalar, wk, EB, i, "wk"))
                kt_2.append(pair_load(nc.scalar, kt, L, i, "kt"))
                wv_2.append(pair_load(nc.gpsimd, wv, EB, i, "wv"))
            # epilogue weight shard [256, 1024] right after inputs (sync FIFO)
            wos_t = [sb.tile([128, D], bf16, tag=f"wos{i}", name=f"wos{i}")
                     for i in range(2)]
            for i in range(2):
                nc.sync.dma_start(wos_t[i][:], wos[128 * i:128 * (i + 1), :])

            qt_t = [qt_2[i // 2][:, L * (i % 2):L * (i % 2 + 1)]
                    for i in range(8)]
            kt_t = [kt_2[i // 2][:, L * (i % 2):L * (i % 2 + 1)]
                    for i in range(8)]
            wq_t = [wq_2[i // 2][:, EB * (i % 2):EB * (i % 2 + 1)]
                    for i in range(8)]
            wk_t = [wk_2[i // 2][:, EB * (i % 2):EB * (i % 2 + 1)]
                    for i in range(8)]
            wv_t = [wv_2[i // 2][:, EB * (i % 2):EB * (i % 2 + 1)]
                    for i in range(8)]

            # ---- phase 1: projections, dc-pipelined, PSUM-resident ----
            qp0 = ps_big.tile([128, 1024], f32, tag="big", name="qp0")
            kp0 = ps_big.tile([128, 1024], f32, tag="big", name="kp0")
            qp1 = [ps_at.tile([128, 512], f32, tag="at", name=f"qp1_{qc}")
                   for qc in range(2)]
            kp1 = [ps_at.tile([128, 512], f32, tag="at", name=f"kp1_{qc}")
                   for qc in range(2)]
            for dc in range(8):
                st = (dc == 0)
                sp = (dc == 7)
                for qc in range(2):
                    qs = slice(512 * qc, 512 * (qc + 1))
                    nc.tensor.matmul(qp0[:, qs], wq_t[dc][:, 0:128],
                                     qt_t[dc][:, qs], start=st, stop=sp)
                for qc in range(2):
                    qs = slice(512 * qc, 512 * (qc + 1))
                    nc.tensor.matmul(qp1[qc][:], wq_t[dc][:, 128:256],
                                     qt_t[dc][:, qs], start=st, stop=sp)
                for qc in range(2):
                    qs = slice(512 * qc, 512 * (qc + 1))
                    nc.tensor.matmul(kp0[:, qs], wk_t[dc][:, 0:128],
                                     kt_t[dc][:, qs], start=st, stop=sp)
                for qc in range(2):
                    qs = slice(512 * qc, 512 * (qc + 1))
                    nc.tensor.matmul(kp1[qc][:], wk_t[dc][:, 128:256],
                                     kt_t[dc][:, qs], start=st, stop=sp)

            # PSUM -> SBUF (inputs pre-zeroed on host, plain copies)
            qpt = [sb.tile([128, L], bf16, tag=f"qpt{i}", name=f"qpt{i}")
                   for i in range(2)]
            kpt = [sb.tile([128, L], bf16, tag=f"kpt{i}", name=f"kpt{i}")
                   for i in range(2)]
            nc.vector.tensor_copy(kpt[0][:, :], kp0[:])
            nc.vector.tensor_copy(qpt[0][:, :], qp0[:])
            for qc in range(2):
                qs = slice(512 * qc, 512 * (qc + 1))
                nc.vector.tensor_copy(kpt[1][:, qs], kp1[qc][:])
                nc.vector.tensor_copy(qpt[1][:, qs], qp1[qc][:])

            # Vp natural [k-tokens, e] with ones column per head (65-stride)
            vpa = [sb.tile([128, 65 * HPC], bf16, tag=f"vpa{i}",
                           name=f"vpa{i}") for i in range(8)]
            for tt in range(8):
                nc.gpsimd.memset(vpa[tt][:], 1.0)

            def vproj(tt):
                pv = ps_at.tile([128, EB], f32, tag="at", name=f"pv{tt}")
                for dc in range(8):
                    nc.tensor.matmul(
                        pv[:], kt_t[dc][:, 128 * tt:128 * (tt + 1)],
                        wv_t[dc][:], start=(dc == 0), stop=(dc == 7))
                # one strided copy per chunk: 4 heads at 65-stride
                dstv = vpa[tt][:].rearrange("p (h m) -> p h m", h=HPC)[
                    :, :, 0:64]
                nc.vector.tensor_copy(dstv, pv[:].rearrange(
                    "p (h m) -> p h m", h=HPC))

            vproj(0)
            vproj(1)

            attn = [sb.tile([128, L], bf16, tag=f"attn{i}", name=f"attn{i}")
                    for i in range(2)]
            ot = [sb.tile([128, L], bf16, tag=f"ot{i}", name=f"ot{i}")
                  for i in range(2)]

            # ---- phase 2+3: attention q-chunk-major; local partial
            # output projection per q-chunk ----
            for qc in range(2):
                qs = slice(512 * qc, 512 * (qc + 1))
                for hp in (0, 2):
                    et = hp // 2
                    ats = []
                    for h in (hp, hp + 1):
                        at = ps_at.tile([65, 512], f32, tag="at",
                                        name=f"at{h}_{qc}")
                        nc.tensor.matmul(at[:], e64_t[:], mvn_t[:, qs],
                                         start=True, stop=False)
                        ats.append(at)
                    # software pipeline: S(k) issued ahead of attn(k-1)
                    p_prev = None
                    for ki in range(8):
                        ks = slice(128 * ki, 128 * (ki + 1))
                        s_ps = ps_big.tile([128, 1024], f32, tag="big",
                                           name=f"s{hp}_{qc}_{ki}")
                        for j in range(2):
                            ro = 64 * j
                            nc.tensor.matmul(
                                s_ps[:, 512 * j:512 * (j + 1)],
                                kpt[et][ro:ro + 64, ks],
                                qpt[et][ro:ro + 64, qs],
                                start=True, stop=True)
                        p_t = rot.tile([128, 1024], bf16, tag="p",
                                       name=f"p{hp}_{qc}_{ki}")
                        nc.scalar.activation(p_t[:], s_ps[:], EXP,
                                             bias=bk_t[:, ki:ki + 1],
                                             scale=1.0 / 32)
                        if qc == 0 and hp == 0 and ki < 6:
                            vproj(ki + 2)
                        if p_prev is not None:
                            kp_, pp = p_prev
                            for j, h in enumerate((hp, hp + 1)):
                                nc.tensor.matmul(
                                    ats[j][:], vpa[kp_][:, 65 * h:65 * h + 65],
                                    pp[:, 512 * j:512 * (j + 1)],
                                    start=False, stop=False)
                        p_prev = (ki, p_t)
                    kp_, pp = p_prev
                    for j, h in enumerate((hp, hp + 1)):
                        nc.tensor.matmul(
                            ats[j][:], vpa[kp_][:, 65 * h:65 * h + 65],
                            pp[:, 512 * j:512 * (j + 1)],
                            start=False, stop=True)
                    # normalize: attn = at[0:64] / denom (denom row 64)
                    # NB: reciprocal_approx_fast is broken on PSUM reads at
                    # partition offset 64 — bounce the row through SBUF.
                    for j, h in enumerate((hp, hp + 1)):
                        ro = 64 * (h % 2)
                        den = rot.tile([1, 512], f32, tag="den")
                        nc.scalar.activation(den[:], ats[j][64:65, :], CPY)
                        rcp = rot.tile([1, 512], f32, tag="rcp")
                        nc.vector.reciprocal_approx_fast(rcp[:], den[:])
                        bc = rot.tile([64, 512], f32, tag="bc")
                        nc.gpsimd.partition_broadcast(bc[:], rcp[:])
                        nc.vector.tensor_mul(
                            attn[et][ro:ro + 64, qs], ats[j][0:64, :], bc[:])
                    # residual; stream O block out (host needs it)
                    nc.vector.tensor_add(ot[et][:, qs], qpt[et][:, qs],
                                         attn[et][:, qs])
                    nc.gpsimd.dma_start(oout[128 * et:128 * (et + 1), qs],
                                        ot[et][:, qs])

                # local partial output projection for this q-chunk:
                # pout[:, qs] = (Wo.T[block] ).T-contract over 256 local feats
                po = sb.tile([128, 4096], bf16, tag=f"po{qc}", name=f"po{qc}")
                for ec in range(8):
                    fpt = ps_at.tile([128, 512], f32, tag="at",
                                     name=f"fp{qc}_{ec}")
                    for dc in range(2):
                        nc.tensor.matmul(
                            fpt[:], wos_t[dc][:, 128 * ec:128 * (ec + 1)],
                            ot[dc][:, qs], start=(dc == 0), stop=(dc == 1))
                    nc.vector.tensor_copy(po[:, 512 * ec:512 * (ec + 1)],
                                          fpt[:])
                dstv = pout[:, qs].rearrange("(ec p) t -> p ec t", p=128)
                srcv = po[:].rearrange("p (ec t) -> p ec t", ec=8)
                nc.sync.dma_start(dstv, srcv)

    nc.compile()
    return nc


def _get_compiled():
    global _compiled
    if _compiled is None:
        _compiled = _build()
    return _compiled


def kernel(Q, K, mask_Q, mask_K, Wq, Wk, Wv, Wo):
    global LAST_RESULTS
    import ml_dtypes
    from concourse.bass_utils import run_bass_kernel_spmd

    bf = ml_dtypes.bfloat16
    Q = np.asarray(Q, np.float32)
    K = np.asarray(K, np.float32)
    mask_Q = np.asarray(mask_Q, bool)
    mask_K = np.asarray(mask_K, bool)
    Wq = np.asarray(Wq, np.float32)
    Wk = np.asarray(Wk, np.float32)
    Wv = np.asarray(Wv, np.float32)
    Wo = np.asarray(Wo, np.float32)

    nc = _get_compiled()

    # host-side masking: zero masked token rows before projection
    Qz = np.where(mask_Q[:, :, None], 0.0, Q)
    Kz = np.where(mask_K[:, :, None], 0.0, K)

    e64v = np.zeros((1, 65), np.float32)
    e64v[0, 64] = BIG
    wot = np.ascontiguousarray(Wo.T.astype(bf))
    in_maps = []
    for c in range(NCORES):
        b, g = c % 2, c // 2
        eb = slice(EB * g, EB * (g + 1))
        bias = np.where(mask_K[b], NEG, 0.0).astype(np.float32)
        in_maps.append({
            "qt": np.ascontiguousarray(Qz[b].T.astype(bf)),
            "kt": np.ascontiguousarray(Kz[b].T.astype(bf)),
            "wq": np.ascontiguousarray(Wq[eb, :].T.astype(bf)),
            "wk": np.ascontiguousarray(Wk[eb, :].T.astype(bf)),
            "wv": np.ascontiguousarray(Wv[eb, :].T.astype(bf)),
            "wos": np.ascontiguousarray(wot[eb, :]),
            "bk": np.ascontiguousarray(bias.reshape(8, 128).T),
            "e64": e64v.astype(bf),
            "mvn": mask_Q[b].astype(bf)[None, :],
        })

    res = run_bass_kernel_spmd(nc, in_maps, core_ids=list(range(NCORES)),
                               trace=TRACE, **TRACE_KWARGS)
    LAST_RESULTS = res

    # unshard: O blocks -> O_full; sum partials per batch; relu + residual
    full = np.empty((B, L, D), np.float32)
    for b in range(B):
        Ob = np.empty((L, D), np.float32)
        ffb = np.zeros((L, D), np.float32)
        for g in range(4):
            c = 2 * g + b
            r = res.results[c]
            Ob[:, EB * g:EB * (g + 1)] = r["oout"].T.astype(np.float32)
            ffb += r["pout"].T.astype(np.float32)
        full[b] = Ob + np.maximum(ffb, 0.0)
    return full


# revision 10
# speedup vs baseline: 1.5407x; 1.0598x over previous
"""Distributed masked-attention kernel for 8 TRN2 NeuronCores (v5).

Reference computation (B=2, L=1024, D=1024, H=16, DH=64):
    Qz, Kz = masked Q, K;  Qp/Kp/Vp = projections (V = K)
    per-head attention with outer-product validity mask, softmax scaled
    by 1/sqrt(D);  O = Qp + attn;  out = O + relu(mask_q(O @ Wo.T))

Sharding: core c = 2*g + b handles batch b = c%2, head group g = c//2
(4 heads, feature block e = [256g, 256g+256)).  Activations are
feature-major ("X.T" = [features, tokens]); host pre-transposes,
pre-zeroes masked rows, and casts to bf16.

v5: no collectives.  Each core emits its O block (for the residual)
and its partial output projection O_c @ Wo[:, block].T; the host sums
the four feature-block partials per batch and applies relu + residual
while unsharding.  This removes the cross-core barrier (~50us of ncfw
startup) and the serial AllToAll chain from the device critical path.

  - projections accumulate in 8 PSUM banks and consume input-DMA
    chunks as they arrive (dc-pipelined).
  - ACT does only exp + denominator-row copies; reciprocals on DVE.
  - partial output projection per q-chunk (16 MMs) overlaps the next
    attention chunk; outputs stream out as they are produced.
"""
import numpy as np

B, L, D = 2, 1024, 1024
H, DH = 16, 64
NCORES = 8
HPC = 4          # heads per core
EB = 256         # feature block per core
NEG = -30000.0   # masked-k bias (exp -> exact 0)
BIG = 1e30       # masked-q denominator prefill

TRACE = False
TRACE_KWARGS = {}
LAST_RESULTS = None

_compiled = None


def _build():
    import concourse.bacc as bacc
    import concourse.tile as tile
    from concourse import mybir

    f32 = mybir.dt.float32
    bf16 = mybir.dt.bfloat16
    f8 = mybir.dt.float8e4
    DR = mybir.MatmulPerfMode.DoubleRow
    EXP = mybir.ActivationFunctionType.Exp
    CPY = mybir.ActivationFunctionType.Copy

    nc = bacc.Bacc("TRN2", target_bir_lowering=False, debug=False,
                   num_devices=NCORES)

    qt = nc.dram_tensor("qt", [D, L], bf16, kind="ExternalInput")
    kt = nc.dram_tensor("kt", [D, L], bf16, kind="ExternalInput")
    wq = nc.dram_tensor("wq", [D, EB], bf16, kind="ExternalInput")
    wk = nc.dram_tensor("wk", [D, EB], bf16, kind="ExternalInput")
    wv = nc.dram_tensor("wv", [D, EB], bf16, kind="ExternalInput")
    wos = nc.dram_tensor("wos", [EB, D], bf16, kind="ExternalInput")
    bk = nc.dram_tensor("bk", [128, 8], f32, kind="ExternalInput")
    e64 = nc.dram_tensor("e64", [1, 65], bf16, kind="ExternalInput")
    mvn = nc.dram_tensor("mvn", [1, L], bf16, kind="ExternalInput")
    pout = nc.dram_tensor("pout", [D, L], bf16, kind="ExternalOutput")
    oout = nc.dram_tensor("oout", [EB, L], bf16, kind="ExternalOutput")

    with tile.TileContext(nc) as tc:
        with (
            tc.tile_pool(name="sb", bufs=1) as sb,
            tc.tile_pool(name="rot", bufs=4) as rot,
            tc.tile_pool(name="ps_big", bufs=2, space="PSUM") as ps_big,
            tc.tile_pool(name="ps_at", bufs=4, space="PSUM") as ps_at,
        ):
            # ---- constants / masks (tiny, on the otherwise-idle SWDGE) ----
            bk_t = sb.tile([128, 8], f32, tag="bk")
            e64_t = sb.tile([1, 65], bf16, tag="e64")
            mvn_t = sb.tile([1, L], bf16, tag="mvn")
            nc.gpsimd.dma_start(bk_t[:], bk[:])
            nc.gpsimd.dma_start(e64_t[:], e64[:])
            nc.gpsimd.dma_start(mvn_t[:], mvn[:])

            # ---- warmup: PE matmuls + ACT exp-table load, no input deps ----
            warm_w = sb.tile([128, 128], bf16, tag="warmw")
            warm_x = sb.tile([128, 512], bf16, tag="warmx")
            nc.vector.memset(warm_w[:], 0.5)
            nc.vector.memset(warm_x[:], 0.5)
            warm_act = rot.tile([128, 512], bf16, tag="p", name="warm_act")
            nc.scalar.activation(warm_act[:], warm_x[:], EXP, scale=0.25)
            warm_ps = ps_big.tile([128, 1024], f32, tag="big", name="warm_ps")
            for w in range(9):
                nc.tensor.matmul(warm_ps[:, 0:512], warm_w[:], warm_x[:],
                                 start=(w == 0), stop=(w == 8))

            # ---- input DMAs: dc-pair chunks on the two HWDGE queues ----
            def pair_load(eng, dram_t, F, i, tagp):
                t2 = sb.tile([128, 2 * F], bf16, tag=f"{tagp}{i}",
                             name=f"{tagp}{i}")
                dst = t2[:].rearrange("p (h t) -> p h t", h=2)
                srcv = dram_t[256 * i:256 * (i + 1), :].rearrange(
                    "(h p) t -> p h t", p=128)
                eng.dma_start(dst, srcv)
                return t2

            qt_2, kt_2, wq_2, wk_2, wv_2 = [], [], [], [], []
            for i in range(4):
                wq_2.append(pair_load(nc.sync, wq, EB, i, "wq"))
                qt_2.append(pair_load(nc.sync, qt, L, i, "qt"))
                wk_2.append(pair_load(nc.scalar, wk, EB, i, "wk"))
                kt_2.append(pair_load(nc.scalar, kt, L, i, "kt"))
                wv_2.append(pair_load(nc.gpsimd, wv, EB, i, "wv"))
            # epilogue weight shard [256, 1024] right after inputs (sync FIFO)
            wos_t = [sb.tile([128, D], bf16, tag=f"wos{i}", name=f"wos{i}")
                     for i in range(2)]
            for i in range(2):
                nc.sync.dma_start(wos_t[i][:], wos[128 * i:128 * (i + 1), :])

            qt_t = [qt_2[i // 2][:, L * (i % 2):L * (i % 2 + 1)]
                    for i in range(8)]
            kt_t = [kt_2[i // 2][:, L * (i % 2):L * (i % 2 + 1)]
                    for i in range(8)]
            wq_t = [wq_2[i // 2][:, EB * (i % 2):EB * (i % 2 + 1)]
                    for i in range(8)]
            wk_t = [wk_2[i // 2][:, EB * (i % 2):EB * (i % 2 + 1)]
                    for i in range(8)]
            wv_t = [wv_2[i // 2][:, EB * (i % 2):EB * (i % 2 + 1)]
                    for i in range(8)]

            # ---- phase 1: projections, dc-pipelined, PSUM-resident ----
            qp0 = ps_big.tile([128, 1024], f32, tag="big", name="qp0")
            kp0 = ps_big.tile([128, 1024], f32, tag="big", name="kp0")
            qp1 = [ps_at.tile([128, 512], f32, tag="at", name=f"qp1_{qc}")
                   for qc in range(2)]
            kp1 = [ps_at.tile([128, 512], f32, tag="at", name=f"kp1_{qc}")
                   for qc in range(2)]
            for dc in range(8):
                st = (dc == 0)
                sp = (dc == 7)
                for qc in range(2):
                    qs = slice(512 * qc, 512 * (qc + 1))
                    nc.tensor.matmul(qp0[:, qs], wq_t[dc][:, 0:128],
                                     qt_t[dc][:, qs], start=st, stop=sp)
                for qc in range(2):
                    qs = slice(512 * qc, 512 * (qc + 1))
                    nc.tensor.matmul(qp1[qc][:], wq_t[dc][:, 128:256],
                                     qt_t[dc][:, qs], start=st, stop=sp)
                for qc in range(2):
                    qs = slice(512 * qc, 512 * (qc + 1))
                    nc.tensor.matmul(kp0[:, qs], wk_t[dc][:, 0:128],
                                     kt_t[dc][:, qs], start=st, stop=sp)
                for qc in range(2):
                    qs = slice(512 * qc, 512 * (qc + 1))
                    nc.tensor.matmul(kp1[qc][:], wk_t[dc][:, 128:256],
                                     kt_t[dc][:, qs], start=st, stop=sp)

            # PSUM -> SBUF (inputs pre-zeroed on host, plain copies)
            qpt = [sb.tile([128, L], bf16, tag=f"qpt{i}", name=f"qpt{i}")
                   for i in range(2)]
            kpt = [sb.tile([128, L], bf16, tag=f"kpt{i}", name=f"kpt{i}")
                   for i in range(2)]
            nc.vector.tensor_copy(kpt[0][:, :], kp0[:])
            nc.vector.tensor_copy(qpt[0][:, :], qp0[:])
            for qc in range(2):
                qs = slice(512 * qc, 512 * (qc + 1))
                nc.vector.tensor_copy(kpt[1][:, qs], kp1[qc][:])
                nc.vector.tensor_copy(qpt[1][:, qs], qp1[qc][:])

            # Vp in fp8 DoubleRow pair layout: vpa8[t][p, h, j, m] holds V
            # for k-token pair-chunks (2t+j); m stride padded to 80 B so the
            # DoubleRow weight AP satisfies step%16==0.  Column m=64 is the
            # softmax-denominator ones column (memset covers it).
            vpa8 = [sb.tile([128, HPC * 2 * 80], f8, tag=f"vpa{i}",
                            name=f"vpa{i}") for i in range(4)]
            for t in range(4):
                nc.gpsimd.memset(vpa8[t][:], 1.0)

            def vproj(tt):
                pv = ps_at.tile([128, EB], f32, tag="at", name=f"pv{tt}")
                for dc in range(8):
                    nc.tensor.matmul(
                        pv[:], kt_t[dc][:, 128 * tt:128 * (tt + 1)],
                        wv_t[dc][:], start=(dc == 0), stop=(dc == 7))
                # one strided fp8 cast per chunk: 4 heads at 80-stride
                dstv = vpa8[tt // 2][:].rearrange(
                    "p (h j m) -> p h j m", h=HPC, j=2)[:, :, tt % 2, 0:64]
                nc.vector.tensor_copy(dstv, pv[:].rearrange(
                    "p (h m) -> p h m", h=HPC))

            vproj(0)
            vproj(1)

            attn = [sb.tile([128, L], bf16, tag=f"attn{i}", name=f"attn{i}")
                    for i in range(2)]
            ot = [sb.tile([128, L], bf16, tag=f"ot{i}", name=f"ot{i}")
                  for i in range(2)]

            # ---- phase 2+3: attention q-chunk-major; local partial
            # output projection per q-chunk ----
            for qc in range(2):
                qs = slice(512 * qc, 512 * (qc + 1))
                for hp in (0, 2):
                    et = hp // 2
                    ats = []
                    for h in (hp, hp + 1):
                        at = ps_at.tile([65, 512], f32, tag="at",
                                        name=f"at{h}_{qc}")
                        nc.tensor.matmul(at[:], e64_t[:], mvn_t[:, qs],
                                         start=True, stop=False)
                        ats.append(at)
                    # software pipeline; exp writes fp8 pair tiles, AV runs
                    # fp8 DoubleRow (contracts 256 k per MM, 2 MMs per pair)
                    def av(t, p2, stop):
                        for jh in range(2):
                            lhsT = vpa8[t][:].rearrange(
                                "p (h j m) -> p h j m", h=HPC, j=2)[
                                :, hp + jh, :, 0:65]
                            rhs = p2[:].rearrange(
                                "p (h j q) -> p h j q", h=2, j=2)[:, jh]
                            nc.tensor.matmul(ats[jh][:], lhsT, rhs,
                                             start=False, stop=stop,
                                             perf_mode=DR)

                    p2_tiles = {}
                    for ki in range(8):
                        t, jj = ki // 2, ki % 2
                        ks = slice(128 * ki, 128 * (ki + 1))
                        s_ps = ps_big.tile([128, 1024], f32, tag="big",
                                           name=f"s{hp}_{qc}_{ki}")
                        for j in range(2):
                            ro = 64 * j
                            nc.tensor.matmul(
                                s_ps[:, 512 * j:512 * (j + 1)],
                                kpt[et][ro:ro + 64, ks],
                                qpt[et][ro:ro + 64, qs],
                                start=True, stop=True)
                        if jj == 0:
                            p2_tiles[t] = rot.tile([128, 2048], f8, tag="p",
                                                   name=f"p{hp}_{qc}_{t}")
                        dst = p2_tiles[t][:].rearrange(
                            "p (h j q) -> p h j q", h=2, j=2)[:, :, jj, :]
                        nc.scalar.activation(dst, s_ps[:], EXP,
                                             bias=bk_t[:, ki:ki + 1],
                                             scale=1.0 / 32)
                        if qc == 0 and hp == 0 and ki < 6:
                            vproj(ki + 2)
                        if jj == 0 and t >= 1:
                            av(t - 1, p2_tiles.pop(t - 1), stop=False)
                    av(3, p2_tiles.pop(3), stop=True)
                    # normalize: attn = at[0:64] / denom (denom row 64)
                    # NB: reciprocal_approx_fast is broken on PSUM reads at
                    # partition offset 64 — bounce the row through SBUF
                    # (alternating ACT/DVE to balance engine load).
                    for j, h in enumerate((hp, hp + 1)):
                        ro = 64 * (h % 2)
                        den = rot.tile([1, 512], f32, tag="den")
                        if j == 0:
                            nc.scalar.activation(den[:], ats[j][64:65, :],
                                                 CPY)
                        else:
                            nc.vector.tensor_copy(den[:], ats[j][64:65, :])
                        rcp = rot.tile([1, 512], f32, tag="rcp")
                        nc.vector.reciprocal_approx_fast(rcp[:], den[:])
                        bc = rot.tile([64, 512], f32, tag="bc")
                        nc.gpsimd.partition_broadcast(bc[:], rcp[:])
                        nc.vector.tensor_mul(
                            attn[et][ro:ro + 64, qs], ats[j][0:64, :], bc[:])
                    # residual; stream O block out (host needs it)
                    nc.vector.tensor_add(ot[et][:, qs], qpt[et][:, qs],
                                         attn[et][:, qs])
                    nc.gpsimd.dma_start(oout[128 * et:128 * (et + 1), qs],
                                        ot[et][:, qs])

                # local partial output projection for this q-chunk:
                # pout[:, qs] = (Wo.T[block] ).T-contract over 256 local feats
                po = sb.tile([128, 4096], bf16, tag=f"po{qc}", name=f"po{qc}")
                for ec in range(8):
                    fpt = ps_at.tile([128, 512], f32, tag="at",
                                     name=f"fp{qc}_{ec}")
                    for dc in range(2):
                        nc.tensor.matmul(
                            fpt[:], wos_t[dc][:, 128 * ec:128 * (ec + 1)],
                            ot[dc][:, qs], start=(dc == 0), stop=(dc == 1))
                    pslice = po[:, 512 * ec:512 * (ec + 1)]
                    if ec % 2:
                        nc.scalar.activation(pslice, fpt[:], CPY)
                    else:
                        nc.vector.tensor_copy(pslice, fpt[:])
                    # stream each 128-row block out as soon as it's copied
                    nc.sync.dma_start(pout[128 * ec:128 * (ec + 1), qs],
                                      pslice)

    nc.compile()
    return nc


def _get_compiled():
    global _compiled
    if _compiled is None:
        _compiled = _build()
    return _compiled


def kernel(Q, K, mask_Q, mask_K, Wq, Wk, Wv, Wo):
    global LAST_RESULTS
    import ml_dtypes
    from concourse.bass_utils import run_bass_kernel_spmd

    bf = ml_dtypes.bfloat16
    Q = np.asarray(Q, np.float32)
    K = np.asarray(K, np.float32)
    mask_Q = np.asarray(mask_Q, bool)
    mask_K = np.asarray(mask_K, bool)
    Wq = np.asarray(Wq, np.float32)
    Wk = np.asarray(Wk, np.float32)
    Wv = np.asarray(Wv, np.float32)
    Wo = np.asarray(Wo, np.float32)

    nc = _get_compiled()

    # host-side masking: zero masked token rows before projection
    Qz = np.where(mask_Q[:, :, None], 0.0, Q)
    Kz = np.where(mask_K[:, :, None], 0.0, K)

    e64v = np.zeros((1, 65), np.float32)
    e64v[0, 64] = BIG
    wot = np.ascontiguousarray(Wo.T.astype(bf))
    in_maps = []
    for c in range(NCORES):
        b, g = c % 2, c // 2
        eb = slice(EB * g, EB * (g + 1))
        bias = np.where(mask_K[b], NEG, 0.0).astype(np.float32)
        in_maps.append({
            "qt": np.ascontiguousarray(Qz[b].T.astype(bf)),
            "kt": np.ascontiguousarray(Kz[b].T.astype(bf)),
            "wq": np.ascontiguousarray(Wq[eb, :].T.astype(bf)),
            "wk": np.ascontiguousarray(Wk[eb, :].T.astype(bf)),
            "wv": np.ascontiguousarray(Wv[eb, :].T.astype(bf)),
            "wos": np.ascontiguousarray(wot[eb, :]),
            "bk": np.ascontiguousarray(bias.reshape(8, 128).T),
            "e64": e64v.astype(bf),
            "mvn": mask_Q[b].astype(bf)[None, :],
        })

    res = run_bass_kernel_spmd(nc, in_maps, core_ids=list(range(NCORES)),
                               trace=TRACE, **TRACE_KWARGS)
    LAST_RESULTS = res

    # unshard: O blocks -> O_full; sum partials per batch; relu + residual
    full = np.empty((B, L, D), np.float32)
    for b in range(B):
        Ob = np.empty((L, D), np.float32)
        ffb = np.zeros((L, D), np.float32)
        for g in range(4):
            c = 2 * g + b
            r = res.results[c]
            Ob[:, EB * g:EB * (g + 1)] = r["oout"].T.astype(np.float32)
            ffb += r["pout"].T.astype(np.float32)
        full[b] = Ob + np.maximum(ffb, 0.0)
    return full
